# revision 30
# baseline (speedup 1.0000x reference)
"""Trainium2 Bass kernel for nn_BasicTransformerBlock (self-attn + cross-attn
+ GEGLU FF, dim=1024, heads=16, seq=4096, ctx=77).

Strategy (8 NeuronCores), v6:
 - Sequence-parallel: each core owns 512 tokens end-to-end, activations kept
   transposed [channel, token] so projections contract over the partition axis.
 - Two combined K+V AllGathers (halves), triggered as soon as their input
   tiles are stored; AV accumulation runs t4-major so it can start when the
   first V half lands.
 - Query-chunk software pipelining (2 x 256): chunk1's ScalarE-bound softmax
   exp stream is interleaved with chunk0's PE-bound o1/cross/FF work. Gelu +
   FF2 of chunk0 are deferred past the exp region (exp and gelu live in
   different ACT table sets; each switch costs ~2.7us).
 - Softmax: no-max-subtraction exp on ScalarE out of PSUM, batched 2 k-tiles
   per ACT instruction (N=1024); denominator rides the AV matmul as a ones
   column; per-pair 1/z broadcast via K=1 selector matmuls.
 - PSUM budget (8 banks): scores 2x[128,4,256] (4) + AV psA/psB (2) +
   work pool (2); z-broadcast rides a scores-pool slot.
"""
import numpy as np
import ml_dtypes
from collections import deque
from contextlib import ExitStack

import concourse.bass as bass
import concourse.tile as tile
import concourse.mybir as mybir
from concourse.bass_utils import run_bass_kernel_spmd


# --- inlined BIR sync-wait legalizer (toolchain accepts max 1 wait/inst) ---
import json as _json


def _legalize_bir_json(raw, max_waits=1):
    d = _json.loads(raw)
    ctr = 0
    for f in d.get("functions", []):
        for bb in f.get("blocks", []):
            out = []
            for ins in bb.get("instructions", []):
                si = ins.get("sync_info")
                if si:
                    waits = si.get("on_wait") or []
                    if len(waits) > max_waits:
                        extra, keep = waits[:-max_waits], waits[-max_waits:]
                        for w in extra:
                            ctr += 1
                            out.append({
                                "debug": ins.get("debug", 0),
                                "engine": ins["engine"],
                                "ins": [],
                                "outs": [],
                                "name": f"waitfix-{ctr}",
                                "opcode": "EventSemaphore",
                                "sync_info": {"on_update": [], "on_wait": [w]},
                            })
                        si["on_wait"] = keep
                    ups = si.get("on_update") or []
                    if len(ups) > 1:
                        raise AssertionError(
                            f"instruction {ins.get('name')} has {len(ups)} updates")
                out.append(ins)
            bb["instructions"] = out
    return _json.dumps(d).encode()


def _install_legalizer(max_waits=1):
    import concourse.bass as _bassmod

    if getattr(_bassmod.Bass, "_legalize_installed", False):
        return
    orig = _bassmod.Bass.to_json_bytes

    def patched(self):
        return _legalize_bir_json(orig(self), max_waits=max_waits)

    _bassmod.Bass.to_json_bytes = patched
    _bassmod.Bass._legalize_installed = True


_install_legalizer()

F32 = mybir.dt.float32
BF16 = mybir.dt.bfloat16
AF = mybir.ActivationFunctionType
OP = mybir.AluOpType

DIM = 1024
HEADS = 16
D = 64
CTX = 768
FF = 4096
T = 4096
NCORES = 8
TO = T // NCORES          # 512 own tokens per core
PAIRS = HEADS // 2        # 8 head pairs
CKT = DIM // 128          # 8 contraction tiles over DIM
CKT_CTX = CTX // 128      # 6 contraction tiles over CTX
TCX = 77
TCXP = 80  # ctx tokens padded
SCALE = D ** -0.5
EPS = 1e-5
CH = 256                  # query chunk width
NCH = TO // CH            # 2 chunks
GRP = 16                  # 2-kt exp groups per pair per chunk

# AllGather payload (bf16 elems, per rank, per half):
#   [K^T m-tiles (4 x 128 x TO) | V-aug token block (256 x V_ROW)]
V_ROW = HEADS * (D + 1)             # 1040: per-token augmented V row
KH = 4 * 128 * TO                   # K half block
VH = 256 * V_ROW                    # V half block (2 t4 tiles)
AGH = KH + VH


def _ap(tensor_ap, offset, steps):
    """Raw AP view on a (flat) dram tensor: steps = [[step, count], ...]."""
    return bass.AP(tensor=tensor_ap.tensor, offset=tensor_ap.offset + offset,
                   ap=list(steps))


def build_nc(fake_ag=False, interleave=True, lag=14, stop_level=99, nb_pairs=8, b_mode="full"):
    nc = bass.Bass(trn_type="TRN2")

    # ---- dram tensors ----------------------------------------------------
    xT = nc.dram_tensor("xT", [DIM, TO], BF16, kind="ExternalInput")
    ctxT = nc.dram_tensor("ctxT", [CTX, TCXP], BF16, kind="ExternalInput")

    def w_in(name, shape, dt=BF16):
        return nc.dram_tensor(name, list(shape), dt, kind="ExternalInput")

    wq1t = w_in("wq1t", (128, 8, CKT, 128))
    wk1t = w_in("wk1t", (128, 8, CKT, 128))
    wv1t = w_in("wv1t", (128, 2, CKT, 512))
    o1t = w_in("o1t", (128, 8, CKT, 128))
    wq2t = w_in("wq2t", (128, 8, CKT, 128))
    k2t = w_in("k2t", (128, 8, CKT_CTX, 128))
    v2t = w_in("v2t", (128, 2, CKT_CTX, 512))
    o2t = w_in("o2t", (128, 8, CKT, 128))
    ff1g = w_in("ff1g", (128, 32, CKT, 128))
    ff1a = w_in("ff1a", (128, 32, CKT, 128))
    ff2t = w_in("ff2t", (128, 8, FF // 128, 128))

    qb1c = w_in("qb1c", (128, 8), F32)
    kb1c = w_in("kb1c", (128, 8), F32)
    vrow = w_in("vrow", (1, DIM), BF16)       # v bias as a row
    o1bc = w_in("o1bc", (128, 8), F32)
    qb2c = w_in("qb2c", (128, 8), F32)
    o2bc = w_in("o2bc", (128, 8), F32)
    fb1c = w_in("fb1c", (128, 64), F32)
    padmask = w_in("padmask", (128, 16), F32)
    ff2bc = w_in("ff2bc", (128, 8), F32)

    outT = nc.dram_tensor("outT", [DIM, TO], F32, kind="ExternalOutput")

    with tile.TileContext(nc) as tc, ExitStack() as top:
        dram = top.enter_context(tc.tile_pool(name="dram", bufs=1, space="DRAM"))
        p_const = top.enter_context(tc.tile_pool(name="p_const", bufs=1))
        p_xin = top.enter_context(tc.tile_pool(name="p_xin", bufs=1))

        # x first on the HWDGE ring: everything in phase A waits on it
        xt_all = p_xin.tile([128, CKT, TO], BF16, name="xt_all")
        nc.sync.dma_start(
            out=xt_all,
            in_=_ap(xT.ap(), 0, [[TO, 128], [128 * TO, CKT], [1, TO]]))

        # ---- constants (gpsimd ring: keep the HWDGE ring clear) ----------
        oneN = p_const.tile([128, 1], BF16, name="oneN")
        nc.vector.memset(oneN[:], 1.0 / DIM)
        ones1r = p_const.tile([1, 128], BF16, name="ones1r")  # K=1 bcast lhsT
        nc.vector.memset(ones1r[:], 1.0)
        ones16 = p_const.tile([128, 16], F32, name="ones16")
        nc.vector.memset(ones16[:], 1.0)
        padones = p_const.tile([128, 16], F32, name="padones")
        nc.gpsimd.dma_start(out=padones[:], in_=padmask.ap())
        eps_row = p_const.tile([1, 1], F32, name="eps_row")
        nc.vector.memset(eps_row[:], EPS)
        selA = p_const.tile([1, 128], BF16, name="selA")
        nc.vector.memset(selA[:], 0.0)
        nc.vector.memset(selA[0:1, 0:64], 1.0)
        selB = p_const.tile([1, 128], BF16, name="selB")
        nc.vector.memset(selB[:], 0.0)
        nc.vector.memset(selB[0:1, 64:128], 1.0)
        vbrow = p_const.tile([1, DIM], BF16, name="vbrow")
        nc.gpsimd.dma_start(out=vbrow[:], in_=vrow.ap())

        def bias_tile(name, dram_t, cols):
            t = p_const.tile([128, cols], F32, name=name)
            nc.gpsimd.dma_start(out=t[:], in_=dram_t.ap())
            return t

        qb1 = bias_tile("qb1", qb1c, 8)
        kb1 = bias_tile("kb1", kb1c, 8)
        o1b = bias_tile("o1b", o1bc, 8)
        qb2 = bias_tile("qb2", qb2c, 8)
        o2b = bias_tile("o2b", o2bc, 8)
        fb1 = bias_tile("fb1", fb1c, 64)
        ff2b = bias_tile("ff2b", ff2bc, 8)

        ctx_sb = []
        for i in range(CKT_CTX):
            t = p_const.tile([128, TCXP], BF16, name=f"ctxsb{i}")
            nc.gpsimd.dma_start(out=t[:], in_=ctxT.ap()[i * 128:(i + 1) * 128, :])
            ctx_sb.append(t)

        xt = [xt_all[:, i, :] for i in range(CKT)]

        # ---- layernorm helper (width W, psum from `ps` pool) -------------
        def layernorm(xtiles, W, out_pool, ps, work, tag):
            """xtiles: 8 sbuf views [128, W] BF16 -> 8 xhat tiles in out_pool."""
            ps_s = ps.tile([1, W], F32, name=f"pss_{tag}", tag="w")
            ps_q = ps.tile([1, W], F32, name=f"psq_{tag}", tag="w")
            for i in range(8):
                sq = work.tile([128, W], BF16, name=f"sq_{tag}", tag="lnsq")
                nc.vector.tensor_tensor(sq[:], xtiles[i][:], xtiles[i][:],
                                        op=OP.mult)
                nc.tensor.matmul(ps_s[:], oneN[:], xtiles[i][:],
                                 start=(i == 0), stop=(i == 7))
                nc.tensor.matmul(ps_q[:], oneN[:], sq[:],
                                 start=(i == 0), stop=(i == 7))
            mu = work.tile([1, W], F32, name=f"mu_{tag}", tag="lnrow")
            nc.vector.tensor_copy(mu[:], ps_s[:])
            var = work.tile([1, W], F32, name=f"var_{tag}", tag="lnrow2")
            nc.vector.tensor_tensor(var[:], mu[:], mu[:], op=OP.mult)
            nc.vector.tensor_tensor(var[:], ps_q[:], var[:], op=OP.subtract)
            lv = work.tile([1, W], F32, name=f"lv_{tag}", tag="lnrow3")
            nc.scalar.activation(lv[:], var[:], AF.Ln, bias=eps_row[:])
            ra = work.tile([1, W], F32, name=f"ra_{tag}", tag="lnrow4")
            nc.scalar.activation(ra[:], lv[:], AF.Exp, scale=-0.5)
            rabf = work.tile([1, W], BF16, name=f"rabf_{tag}", tag="lnrow5")
            nc.vector.tensor_copy(rabf[:], ra[:])
            nmubf = work.tile([1, W], BF16, name=f"nmubf_{tag}", tag="lnrow6")
            nc.vector.tensor_scalar(nmubf[:], mu[:], -1.0, None, op0=OP.mult)
            ps_ra = ps.tile([128, W], F32, name=f"psra_{tag}", tag="w")
            nc.tensor.matmul(ps_ra[:], ones1r[:], rabf[:], start=True, stop=True)
            ps_nm = ps.tile([128, W], F32, name=f"psnm_{tag}", tag="w")
            nc.tensor.matmul(ps_nm[:], ones1r[:], nmubf[:], start=True, stop=True)
            RA = work.tile([128, W], BF16, name=f"RA_{tag}", tag="lnRA")
            nc.vector.tensor_copy(RA[:], ps_ra[:])
            NMU = work.tile([128, W], BF16, name=f"NMU_{tag}", tag="lnNMU")
            nc.vector.tensor_copy(NMU[:], ps_nm[:])
            out = []
            for i in range(8):
                tmp = work.tile([128, W], BF16, name=f"tmp_{tag}", tag="lntmp")
                nc.vector.tensor_tensor(tmp[:], xtiles[i][:], NMU[:], op=OP.add)
                h = out_pool.tile([128, W], BF16, name=f"h_{tag}{i}")
                nc.vector.tensor_tensor(h[:], tmp[:], RA[:], op=OP.mult)
                out.append(h)
            return out

        # ---- streamed projection helper ----------------------------------
        def proj_stream(wdram, rhs_tiles, W, nkt, out_pool, ps, wpool, tag,
                        bias=None, residual=None, res_bias=None,
                        out_dtype=BF16, cast_pool=None):
            outs, casts = [], []
            for m in range(8):
                wm = wpool.tile([128, nkt, 128], BF16, name=f"wm_{tag}",
                                tag="w")
                nc.gpsimd.dma_start(out=wm[:], in_=wdram.ap()[:, m, :, :])
                psy = ps.tile([128, W], F32, name=f"psy_{tag}", tag="w")
                for kt in range(nkt):
                    nc.tensor.matmul(psy[:], wm[:, kt, :], rhs_tiles[kt][:],
                                     start=(kt == 0), stop=(kt == nkt - 1))
                o = out_pool.tile([128, W], out_dtype, name=f"o_{tag}{m}")
                if residual is not None:
                    nc.vector.scalar_tensor_tensor(
                        o[:], psy[:], res_bias[:, m:m + 1],
                        residual[m][:], op0=OP.add, op1=OP.add)
                elif bias is not None:
                    nc.vector.tensor_scalar(o[:], psy[:], bias[:, m:m + 1],
                                            None, op0=OP.add)
                else:
                    nc.vector.tensor_copy(o[:], psy[:])
                outs.append(o)
                if cast_pool is not None:
                    cc = cast_pool.tile([128, W], BF16, name=f"c_{tag}{m}")
                    nc.vector.tensor_copy(cc[:], o[:])
                    casts.append(cc)
            if cast_pool is not None:
                return outs, casts
            return outs

        # ---- AG buffers: two combined K+V halves -------------------------
        ag_in = [dram.tile([AGH], BF16, name=f"ag_in{h}") for h in range(2)]
        ag_space = "Local" if fake_ag else "Shared"
        ag_out = [dram.tile([NCORES * AGH], BF16, name=f"ag_out{h}",
                            addr_space=ag_space) for h in range(2)]

        def ag_half(h):
            src_ap = _ap(ag_in[h][:], 0, [[TO, AGH // TO], [1, TO]])
            if fake_ag:
                for r in range(NCORES):
                    nc.sync.dma_start(
                        out=_ap(ag_out[h][:], r * AGH,
                                [[TO, AGH // TO], [1, TO]]),
                        in_=src_ap)
            else:
                nc.gpsimd.collective_compute(
                    "AllGather", OP.bypass,
                    replica_groups=[list(range(NCORES))],
                    ins=[src_ap], outs=[ag_out[h][:]])

        # ---- persistent pools (created first so transient pools can pop
        # in LIFO order). PSUM: ps_ss 4 + ps_av 2 + ps_wk 2 = 8 banks. ----
        p_QT = top.enter_context(tc.tile_pool(name="p_QT", bufs=1))
        p_kv2 = top.enter_context(tc.tile_pool(name="p_kv2", bufs=1))
        p_lnw2 = top.enter_context(tc.tile_pool(name="p_lnw2", bufs=2))
        ps_ss = top.enter_context(tc.tile_pool(name="ps_ss", bufs=2,
                                               space="PSUM"))
        ps_av = top.enter_context(tc.tile_pool(name="ps_av", bufs=2,
                                               space="PSUM"))
        ps_wk = top.enter_context(tc.tile_pool(name="ps_wk", bufs=2,
                                               space="PSUM"))
        p_kp = top.enter_context(tc.tile_pool(name="p_kp", bufs=2))
        p_pt = top.enter_context(tc.tile_pool(name="p_pt", bufs=12))
        p_vp = top.enter_context(tc.tile_pool(name="p_vp", bufs=3))
        p_rb = top.enter_context(tc.tile_pool(name="p_rb", bufs=2))
        p_lw = top.enter_context(tc.tile_pool(name="p_lw", bufs=4))
        p_lw2 = top.enter_context(tc.tile_pool(name="p_lw2", bufs=2))
        p_OT = [top.enter_context(tc.tile_pool(name=f"p_OT{c}", bufs=1))
                for c in range(NCH)]

        # ================= phase A ========================================
        sA2 = ExitStack()   # pools freed once Q/K2/V2 are emitted
        p_h1 = sA2.enter_context(tc.tile_pool(name="p_h1", bufs=1))
        # wq on the gpsimd ring early (needed ~Q-proj time)
        p_wq = sA2.enter_context(tc.tile_pool(name="p_wq", bufs=1))
        wq_sb = p_wq.tile([128, 8, CKT, 128], BF16, name="wq_sb")
        nc.gpsimd.dma_start(out=wq_sb, in_=wq1t.ap())

        with ExitStack() as phA:
            p_wkv = phA.enter_context(tc.tile_pool(name="p_wkv", bufs=1))
            wk_sb = p_wkv.tile([128, 8, CKT, 128], BF16, name="wk_sb")
            nc.sync.dma_start(out=wk_sb, in_=wk1t.ap())
            wv_sb = p_wkv.tile([128, 2, CKT, 512], BF16, name="wv_sb")
            nc.sync.dma_start(out=wv_sb, in_=wv1t.ap())
            h1 = layernorm(xt, TO, p_h1, ps_wk, p_lnw2, "ln1")

            p_st = phA.enter_context(tc.tile_pool(name="p_st", bufs=2))

            def k_proj(ms, h):
                for m in ms:
                    psy = ps_wk.tile([128, TO], F32, name="psy_k1", tag="w")
                    for kt in range(CKT):
                        nc.tensor.matmul(psy[:], wk_sb[:, m, kt, :], h1[kt][:],
                                         start=(kt == 0), stop=(kt == CKT - 1))
                    ko = p_st.tile([128, TO], BF16, name="ko_k1", tag="ko")
                    nc.vector.tensor_scalar(ko[:], psy[:], kb1[:, m:m + 1],
                                            None, op0=OP.add)
                    nc.sync.dma_start(
                        out=_ap(ag_in[h][:], (m % 4) * 128 * TO,
                                [[TO, 128], [1, TO]]),
                        in_=ko[:])

            def v_proj(t4s, h):
                for t4 in t4s:
                    vag = p_st.tile([128, V_ROW], BF16, name="vag", tag="vag")
                    vag3 = vag.rearrange("p (h e) -> p h e", e=D + 1)
                    for nb in range(2):
                        psv = ps_wk.tile([128, 512], F32, name="psv", tag="w")
                        for kt in range(CKT):
                            nc.tensor.matmul(
                                psv[:], h1[kt][:, t4 * 128:(t4 + 1) * 128],
                                wv_sb[:, nb, kt, :],
                                start=(kt == 0), stop=False)
                        nc.tensor.matmul(
                            psv[:], ones1r[:],
                            vbrow[:, nb * 512:(nb + 1) * 512],
                            start=False, stop=True)
                        nc.vector.tensor_copy(
                            vag3[:, nb * 8:(nb + 1) * 8, 0:D],
                            psv[:].rearrange("p (h e) -> p h e", e=D))
                    nc.scalar.copy(vag3[:, :, D:D + 1], ones16.unsqueeze(2))
                    nc.sync.dma_start(
                        out=_ap(ag_in[h][:], KH + (t4 % 2) * 128 * V_ROW,
                                [[V_ROW, 128], [1, V_ROW]]),
                        in_=vag[:])

            k_proj(range(0, 4), 0)
            v_proj((0, 1), 0)
            ag_half(0)
            k_proj(range(4, 8), 1)
            v_proj((2, 3), 1)
            ag_half(1)

        # deferred weight loads (gpsimd ring, after AG inputs are queued)
        p_kv2w = sA2.enter_context(tc.tile_pool(name="p_kv2w", bufs=1))
        k2_sb = p_kv2w.tile([128, 8, CKT_CTX, 128], BF16, name="k2_sb")
        nc.gpsimd.dma_start(out=k2_sb, in_=k2t.ap())
        v2_sb = p_kv2w.tile([128, 2, CKT_CTX, 512], BF16, name="v2_sb")
        nc.gpsimd.dma_start(out=v2_sb, in_=v2t.ap())

        # Q projection (full width) + cross-attn K2/V2
        QT = []
        for m in range(8):
            psy = ps_wk.tile([128, TO], F32, name="psy_q1", tag="w")
            for kt in range(CKT):
                nc.tensor.matmul(psy[:], wq_sb[:, m, kt, :], h1[kt][:],
                                 start=(kt == 0), stop=(kt == CKT - 1))
            q = p_QT.tile([128, TO], BF16, name=f"qt{m}")
            nc.vector.tensor_scalar(q[:], psy[:], qb1[:, m:m + 1],
                                    None, op0=OP.add)
            QT.append(q)

        K2T = []
        for m in range(8):
            psy = ps_wk.tile([128, TCXP], F32, name="psy_k2", tag="w")
            for kt in range(CKT_CTX):
                nc.tensor.matmul(psy[:], k2_sb[:, m, kt, :], ctx_sb[kt][:],
                                 start=(kt == 0), stop=(kt == CKT_CTX - 1))
            k2 = p_kv2.tile([128, TCXP], BF16, name=f"k2_{m}")
            nc.vector.tensor_copy(k2[:], psy[:])
            K2T.append(k2)

        v2ag = p_kv2.tile([TCXP, V_ROW], BF16, name="v2ag")
        v2ag3 = v2ag.rearrange("p (h e) -> p h e", e=D + 1)
        for nb in range(2):
            psv = ps_wk.tile([TCXP, 512], F32, name="psv2", tag="w")
            for kt in range(CKT_CTX):
                nc.tensor.matmul(psv[:], ctx_sb[kt][:], v2_sb[:, nb, kt, :],
                                 start=(kt == 0), stop=(kt == CKT_CTX - 1))
            nc.vector.tensor_copy(
                v2ag3[:, nb * 8:(nb + 1) * 8, 0:D],
                psv[:].rearrange("p (h e) -> p h e", e=D))
        nc.scalar.copy(v2ag3[:, :, D:D + 1], padones[0:TCXP, :].unsqueeze(2))
        sA2.close()   # free h1 / wq / k2,v2 weight SBUF

        def dump_and_stop(tiles):
            for m in range(8):
                t = tiles[m % len(tiles)]
                o = p_lnw2.tile([128, t.shape[-1]], F32, name="dmp", tag="of")
                nc.vector.tensor_copy(o[:], t[:])
                nc.sync.dma_start(
                    out=outT.ap()[m * 128:(m + 1) * 128, 0:t.shape[-1]],
                    in_=o[:])

        if stop_level <= 1:
            dump_and_stop(QT)
            return nc

        # ================= self-attention chunk emitter ===================
        def attn_pair(c, p, lag):
            """Emit scores+exp+AV for pair p, query chunk c. lag>0 delays AV
            emission by `lag` groups (deque) to keep the PE FIFO clear of
            V-gather waits; returns leftover AV thunks if lag (caller must
            flush)."""
            qs = slice(c * CH, (c + 1) * CH)
            h = p // 4
            kpair = p_kp.tile([128, NCORES, TO], BF16, name="kpair", tag="kp")
            nc.sync.dma_start(
                out=kpair,
                in_=_ap(ag_out[h][:], (p % 4) * 128 * TO,
                        [[TO, 128], [AGH, NCORES], [1, TO]]))
            psA = ps_av.tile([128, CH], F32, name="psA", tag="av")
            psB = ps_av.tile([128, CH], F32, name="psB", tag="av")
            state = {"vp8": None}
            pend = deque()

            def scores_exp(g):
                # bank-safe layout: j = head*2 + ktl.  Bank 0 holds head A
                # (kt0, kt1 -- sequential row-group-0 MMs), bank 1 head B.
                # The concurrent tile-position pair writes different banks;
                # within a bank the first MM start=True clears it, the
                # second overwrites its (cleared) half with start=False.
                t4 = g // 4
                pss = ps_ss.tile([128, 4, CH], F32, name="pss", tag="s")
                for ktl in range(2):
                    r = 2 * (g % 4) + ktl
                    blk = slice(t4 * 128, (t4 + 1) * 128)
                    nc.tensor.matmul(pss[:, ktl, :],
                                     kpair[0:64, r, blk], QT[p][0:64, qs],
                                     start=(ktl == 0), stop=(ktl == 1),
                                     tile_position=(0, 0),
                                     skip_group_check=True)
                    nc.tensor.matmul(pss[:, 2 + ktl, :],
                                     kpair[64:128, r, blk], QT[p][64:128, qs],
                                     start=(ktl == 0), stop=(ktl == 1),
                                     tile_position=(64, 0),
                                     skip_group_check=True)
                pt = p_pt.tile([128, 4, CH], BF16, name="pt", tag="pt")
                nc.scalar.activation(pt[:], pss[:], AF.Exp)
                return pt

            def av(g, pt):
                t4 = g // 4
                if g % 4 == 0:
                    vp8 = p_vp.tile([128, NCORES, 2 * (D + 1)], BF16,
                                    name="vp8", tag="vp")
                    nc.sync.dma_start(
                        out=vp8,
                        in_=_ap(ag_out[t4 // 2][:],
                                KH + (t4 % 2) * 128 * V_ROW + p * 2 * (D + 1),
                                [[V_ROW, 128], [AGH, NCORES],
                                 [1, 2 * (D + 1)]]))
                    state["vp8"] = vp8
                vp8 = state["vp8"]
                first, last = (g == 0), (g == GRP - 1)
                for ktl in range(2):
                    r = 2 * (g % 4) + ktl
                    nc.tensor.matmul(psA[0:D + 1, :], vp8[:, r, 0:D + 1],
                                     pt[:, ktl, :],
                                     start=(first and ktl == 0),
                                     stop=(last and ktl == 1))
                    nc.tensor.matmul(psB[0:D + 1, :],
                                     vp8[:, r, D + 1:2 * (D + 1)],
                                     pt[:, 2 + ktl, :],
                                     start=(first and ktl == 0),
                                     stop=(last and ktl == 1))

            for g in range(GRP):
                pt = scores_exp(g)
                if b_mode == "scores":
                    continue
                pend.append((g, pt))
                while len(pend) > lag:
                    gg, pp = pend.popleft()
                    av(gg, pp)
            if not lag and b_mode != "scores":
                assert not pend

            def finish():
                ot = p_OT[c].tile([128, CH], BF16, name=f"ot{c}_{p}")
                if b_mode != "full":
                    nc.vector.memset(ot[:], 0.0)
                    return ot
                nc.vector.tensor_copy(ot[0:64, :], psA[0:D, :])
                zabf = p_rb.tile([1, CH], BF16, name="zabf", tag="za")
                nc.vector.tensor_copy(zabf[:], psA[D:D + 1, :])
                nc.vector.tensor_copy(ot[64:128, :], psB[0:D, :])
                zbbf = p_rb.tile([1, CH], BF16, name="zbbf", tag="zb")
                nc.vector.tensor_copy(zbbf[:], psB[D:D + 1, :])
                psbc = ps_ss.tile([128, CH], F32, name="psbc", tag="s")
                nc.tensor.matmul(psbc[:], selA[:], zabf[:], start=True,
                                 stop=False)
                nc.tensor.matmul(psbc[:], selB[:], zbbf[:], start=False,
                                 stop=True)
                rec = p_rb.tile([128, CH], F32, name="rec", tag="bcs")
                nc.vector.reciprocal(rec[:], psbc[:])
                nc.vector.tensor_tensor(ot[:], ot[:], rec[:], op=OP.mult)
                return ot

            return pend, av, finish

        # ================= downstream (o1..ff1) emitter ===================
        def downstream_steps(c, ctx_out):
            """Generator of emit-steps for chunk c: o1 .. ff1 (raw stash).
            ctx_out collects tensors needed by the tail (ff2 etc)."""
            qs = slice(c * CH, (c + 1) * CH)
            xt_c = [xt_all[:, i, qs] for i in range(CKT)]

            p_x2 = ctx_out["p_x2"]
            p_x3 = ctx_out["p_x3"]
            p_x3b = ctx_out["p_x3b"]
            p_h23 = ctx_out["p_h23"]
            p_gr = ctx_out["p_gr"]

            x2b = proj_stream(o1t, ctx_out["OT"], CH, CKT, p_x2, ps_wk,
                              p_lw, f"o1c{c}", residual=xt_c, res_bias=o1b)
            yield "o1"

            h2 = layernorm(x2b, CH, p_h23, ps_wk, p_lnw2, f"ln2c{c}")
            Q2T = proj_stream(wq2t, h2, CH, CKT, ctx_out["p_q2"], ps_wk,
                              p_lw, f"q2c{c}", bias=qb2)
            yield "ln2q2"

            # cross-attention
            OT2 = []
            for p in range(PAIRS):
                pss = ps_ss.tile([TCXP, 2, 512], F32, name="pss2", tag="s")
                nc.tensor.matmul(pss[:, 0, 0:CH], K2T[p][0:64, :],
                                 Q2T[p][0:64, :],
                                 start=True, stop=True, tile_position=(0, 0))
                nc.tensor.matmul(pss[:, 1, 0:CH], K2T[p][64:128, :],
                                 Q2T[p][64:128, :],
                                 start=True, stop=True, tile_position=(64, 0))
                pt = p_pt.tile([TCXP, 2, CH], BF16, name="pt2", tag="pt")
                nc.scalar.activation(pt[:], pss[:, :, 0:CH], AF.Exp)
                psA2 = ps_wk.tile([128, CH], F32, name="psA2", tag="w")
                psB2 = ps_wk.tile([128, CH], F32, name="psB2", tag="w")
                nc.tensor.matmul(psA2[0:D + 1, :],
                                 v2ag[:, (2 * p) * (D + 1):(2 * p + 1) * (D + 1)],
                                 pt[:, 0, :], start=True, stop=True)
                nc.tensor.matmul(psB2[0:D + 1, :],
                                 v2ag[:, (2 * p + 1) * (D + 1):(2 * p + 2) * (D + 1)],
                                 pt[:, 1, :], start=True, stop=True)
                ot = ctx_out["p_OT2"].tile([128, CH], BF16, name=f"ot2c{c}_{p}")
                nc.vector.tensor_copy(ot[0:64, :], psA2[0:D, :])
                zabf = p_rb.tile([1, CH], BF16, name="zabf2", tag="za")
                nc.vector.tensor_copy(zabf[:], psA2[D:D + 1, :])
                nc.vector.tensor_copy(ot[64:128, :], psB2[0:D, :])
                zbbf = p_rb.tile([1, CH], BF16, name="zbbf2", tag="zb")
                nc.vector.tensor_copy(zbbf[:], psB2[D:D + 1, :])
                psbc = ps_wk.tile([128, CH], F32, name="psbc2", tag="w")
                nc.tensor.matmul(psbc[:], selA[:], zabf[:], start=True,
                                 stop=False)
                nc.tensor.matmul(psbc[:], selB[:], zbbf[:], start=False,
                                 stop=True)
                rec = p_rb.tile([128, CH], F32, name="rec2", tag="bcs")
                nc.vector.reciprocal(rec[:], psbc[:])
                nc.vector.tensor_tensor(ot[:], ot[:], rec[:], op=OP.mult)
                OT2.append(ot)
            yield "cross"

            x3f, x3b = proj_stream(o2t, OT2, CH, CKT, p_x3, ps_wk, p_lw,
                                   f"o2c{c}", residual=x2b, res_bias=o2b,
                                   out_dtype=F32, cast_pool=p_x3b)
            ctx_out["x3f"] = x3f
            h3 = layernorm(x3b, CH, p_h23, ps_wk, p_lnw2, f"ln3c{c}")
            yield "o2ln3"

            # ff1: matmuls + raw stash (gelu deferred out of the exp region)
            graw, araw = [], []
            for i in range(32):
                wg = p_lw.tile([128, CKT, 128], BF16, name="wg_ff1", tag="w")
                nc.gpsimd.dma_start(out=wg[:], in_=ff1g.ap()[:, i, :, :])
                psg = ps_wk.tile([128, CH], F32, name="psg", tag="w")
                for kt in range(CKT):
                    nc.tensor.matmul(psg[:], wg[:, kt, :], h3[kt][:],
                                     start=(kt == 0), stop=(kt == CKT - 1))
                g = p_gr.tile([128, CH], BF16, name=f"graw{i}")
                nc.vector.tensor_copy(g[:], psg[:])
                graw.append(g)
                wa = p_lw.tile([128, CKT, 128], BF16, name="wa_ff1", tag="w")
                nc.gpsimd.dma_start(out=wa[:], in_=ff1a.ap()[:, i, :, :])
                psa = ps_wk.tile([128, CH], F32, name="psa", tag="w")
                for kt in range(CKT):
                    nc.tensor.matmul(psa[:], wa[:, kt, :], h3[kt][:],
                                     start=(kt == 0), stop=(kt == CKT - 1))
                a = p_gr.tile([128, CH], BF16, name=f"araw{i}")
                nc.vector.tensor_scalar(a[:], psa[:], fb1[:, i:i + 1],
                                        None, op0=OP.add)
                araw.append(a)
                if i % 8 == 5 and i < 30:
                    yield f"ff1_{i}"
            ctx_out["graw"] = graw
            ctx_out["araw"] = araw
            yield "ff1_done"

        def tail_gelu_ff2(c, ctx_out):
            """gelu + gate-mult + ff2 + output DMA for chunk c."""
            qs = slice(c * CH, (c + 1) * CH)
            graw, araw, x3f = ctx_out["graw"], ctx_out["araw"], ctx_out["x3f"]
            hT = graw     # gate-multiply lands in graw[i] (it becomes hT[i])
            for i in range(32):
                g = p_lnw2.tile([128, CH], F32, name="gact", tag="gact")
                nc.scalar.activation(g[:], graw[i][:], AF.Gelu,
                                     bias=fb1[:, 32 + i:33 + i], scale=1.0)
                nc.vector.tensor_tensor(graw[i][:], araw[i][:], g[:],
                                        op=OP.mult)
            for m in range(8):
                wm = p_lw2.tile([128, FF // 128, 128], BF16, name="wm_ff2",
                                tag="wff2")
                nc.gpsimd.dma_start(out=wm[:], in_=ff2t.ap()[:, m, :, :])
                psy = ps_wk.tile([128, CH], F32, name="psy_ff2", tag="w")
                for kt in range(FF // 128):
                    nc.tensor.matmul(psy[:], wm[:, kt, :], hT[kt][:],
                                     start=(kt == 0),
                                     stop=(kt == FF // 128 - 1))
                o = p_lnw2.tile([128, CH], F32, name="of", tag="of")
                nc.vector.scalar_tensor_tensor(o[:], psy[:], ff2b[:, m:m + 1],
                                               x3f[m][:],
                                               op0=OP.add, op1=OP.add)
                nc.sync.dma_start(out=outT.ap()[m * 128:(m + 1) * 128, qs],
                                  in_=o[:])

        # ---- chunk context pools -----------------------------------------
        def make_ctx(c, stack):
            return {
                "p_x2": stack.enter_context(tc.tile_pool(name=f"p_x2c{c}", bufs=1)),
                "p_x3": stack.enter_context(tc.tile_pool(name=f"p_x3c{c}", bufs=1)),
                "p_x3b": stack.enter_context(tc.tile_pool(name=f"p_x3bc{c}", bufs=1)),
                "p_h23": stack.enter_context(tc.tile_pool(name=f"p_h23c{c}", bufs=1)),
                "p_q2": stack.enter_context(tc.tile_pool(name=f"p_q2c{c}", bufs=1)),
                "p_OT2": stack.enter_context(tc.tile_pool(name=f"p_OT2c{c}", bufs=1)),
                "p_gr": stack.enter_context(tc.tile_pool(name=f"p_grc{c}", bufs=1)),
                "OT": None,
            }

        # ================= B(c0) with AV lag ==============================
        LAG = lag
        OT0 = []
        carry = None
        for p in range(min(PAIRS, nb_pairs)):
            lag = LAG if p < 2 else 0
            if carry is not None:
                cpend, cav, cfin = carry
                while cpend:
                    gg, pp = cpend.popleft()
                    cav(gg, pp)
                OT0.append(cfin())
                carry = None
            pend, avfn, finfn = attn_pair(0, p, lag)
            if pend:
                carry = (pend, avfn, finfn)
            else:
                OT0.append(finfn())
        assert carry is None

        if stop_level <= 2:
            dump_and_stop(OT0)
            return nc

        # ================= interleave: B(c1) || downstream(c0) ============
        with ExitStack() as sC0, ExitStack() as sC1:
            ctx0 = make_ctx(0, sC0)
            ctx0["OT"] = OT0
            OT1 = []
            if interleave:
                gen0 = downstream_steps(0, ctx0)
                for p in range(PAIRS):
                    pend, avfn, finfn = attn_pair(1, p, 0)
                    assert not pend
                    OT1.append(finfn())
                    try:
                        next(gen0)
                    except StopIteration:
                        pass
                for _ in gen0:
                    pass
            else:
                for _ in downstream_steps(0, ctx0):
                    pass

            # ================= tail ======================================
            tail_gelu_ff2(0, ctx0)
            if not interleave:
                for p in range(PAIRS):
                    pend, avfn, finfn = attn_pair(1, p, 0)
                    assert not pend
                    OT1.append(finfn())
            sC0.close()

            ctx1 = make_ctx(1, sC1)
            ctx1["OT"] = OT1
            for _ in downstream_steps(1, ctx1):
                pass
            tail_gelu_ff2(1, ctx1)

    return nc


# ---------------------------------------------------------------------------
# host side
# ---------------------------------------------------------------------------
def _tile_lhs(w, nm, nkt):
    """[K, M] -> [128, nm, nkt, 128] with [p][m][kt][n] = w[kt*128+p, m*128+n]."""
    K, M = w.shape
    assert K == nkt * 128 and M == nm * 128
    return np.ascontiguousarray(
        w.reshape(nkt, 128, nm, 128).transpose(1, 2, 0, 3))


def _tile_rhs(w, nkt):
    """[K, N] -> [128, N//512, nkt, 512] with [p][nb][kt][n] = w[kt*128+p, nb*512+n]."""
    K, N = w.shape
    assert K == nkt * 128 and N % 512 == 0
    return np.ascontiguousarray(
        w.reshape(nkt, 128, N // 512, 512).transpose(1, 2, 0, 3))


def _bias_cols(b, ncols):
    return np.ascontiguousarray(np.asarray(b, np.float32).reshape(ncols, 128).T)


_NC_CACHE = None


def kernel(**inputs):
    global _NC_CACHE
    inp = {k: np.asarray(v, np.float32) for k, v in inputs.items()}

    x = inp["x"][0]                    # [T, DIM]
    ctx = inp["context"][0]            # [77, CTX]
    xT_full = np.ascontiguousarray(x.T)
    ctxT = np.zeros((CTX, TCXP), np.float32)
    ctxT[:, :TCX] = ctx.T

    wq1 = np.ascontiguousarray((inp["n1_w"][:, None] * inp["q1_w"]) * SCALE)
    wk1 = np.ascontiguousarray(inp["n1_w"][:, None] * inp["k1_w"])
    wv1 = np.ascontiguousarray(inp["n1_w"][:, None] * inp["v1_w"])
    qb1 = (inp["n1_b"] @ inp["q1_w"]) * SCALE
    kb1 = inp["n1_b"] @ inp["k1_w"]
    vb1 = inp["n1_b"] @ inp["v1_w"]
    wq2 = np.ascontiguousarray((inp["n2_w"][:, None] * inp["q2_w"]) * SCALE)
    qb2 = (inp["n2_b"] @ inp["q2_w"]) * SCALE
    ff1 = np.ascontiguousarray(inp["n3_w"][:, None] * inp["ff1_w"])
    fb1 = inp["n3_b"] @ inp["ff1_w"] + inp["ff1_b"]

    shared = {
        "ctxT": ctxT,
        "wq1t": _tile_lhs(wq1, 8, CKT),
        "wk1t": _tile_lhs(wk1, 8, CKT),
        "wv1t": _tile_rhs(wv1, CKT),
        "o1t": _tile_lhs(np.ascontiguousarray(inp["o1_w"]), 8, CKT),
        "wq2t": _tile_lhs(wq2, 8, CKT),
        "k2t": _tile_lhs(np.ascontiguousarray(inp["k2_w"]), 8, CKT_CTX),
        "v2t": _tile_rhs(np.ascontiguousarray(inp["v2_w"]), CKT_CTX),
        "o2t": _tile_lhs(np.ascontiguousarray(inp["o2_w"]), 8, CKT),
        "ff1g": _tile_lhs(np.ascontiguousarray(ff1[:, FF:]), 32, CKT),
        "ff1a": _tile_lhs(np.ascontiguousarray(ff1[:, :FF]), 32, CKT),
        "ff2t": _tile_lhs(np.ascontiguousarray(inp["ff2_w"]), 8, FF // 128),
        "vrow": np.ascontiguousarray(vb1.reshape(1, DIM)),
        "qb1c": _bias_cols(qb1, 8),
        "kb1c": _bias_cols(kb1, 8),
        "o1bc": _bias_cols(inp["o1_b"], 8),
        "qb2c": _bias_cols(qb2, 8),
        "o2bc": _bias_cols(inp["o2_b"], 8),
        "fb1c": _bias_cols(fb1, 64),
        "padmask": np.ascontiguousarray(
            (np.arange(128)[:, None] < TCX).astype(np.float32)
            * np.ones((1, 16), np.float32)),
        "ff2bc": _bias_cols(inp["ff2_b"], 8),
    }
    f32_keys = {"qb1c", "kb1c", "o1bc", "qb2c", "o2bc", "fb1c",
                "ff2bc", "padmask"}

    def _dt(k):
        return np.float32 if k in f32_keys else ml_dtypes.bfloat16

    shared = {k: np.ascontiguousarray(v, dtype=_dt(k))
              for k, v in shared.items()}

    in_maps = []
    for c in range(NCORES):
        m = dict(shared)
        m["xT"] = np.ascontiguousarray(
            xT_full[:, c * TO:(c + 1) * TO], dtype=ml_dtypes.bfloat16)
        in_maps.append(m)

    if _NC_CACHE is None:
        _NC_CACHE = build_nc()
    nc = _NC_CACHE

    res = run_bass_kernel_spmd(nc, in_maps, core_ids=list(range(NCORES)))

    outs = [res.results[c]["outT"].T for c in range(NCORES)]   # each [TO, DIM]
    return np.ascontiguousarray(np.concatenate(outs, axis=0))[None].astype(np.float32)


if __name__ == "__main__":
    d = np.load("/tmp/ref_inputs.npz")
    out = kernel(**{k: d[k] for k in d.files})
    ref = np.load("/tmp/ref_out.npy")
    err = np.abs(out - ref).max()
    print("max abs err:", err, " absmax ref:", np.abs(ref).max(),
          " rel:", err / np.abs(ref).max())
